# revision 1
# baseline (speedup 1.0000x reference)
"""Trainium2 Bass kernel for ChameleonVQVAEEncoderAttnBlock.

Reference computation (per batch b of 16, C=512 channels, N=32*32=1024 spatial):
    h  = GroupNorm32(x) * gamma + beta
    q, k, v = wq@h+bq, wk@h+bk, wv@h+bv          (1x1 convs == channel matmuls)
    S[i,j] = sum_c q[c,i] k[c,j] / sqrt(C)
    A = softmax_j(S)
    o[c,i] = sum_j v[c,j] A[i,j]
    y = wo@o + bo + x

Sharding: pure data parallel, batch 16 -> 2 batches on each of 8 cores.

Per-core kernel strategy (all matmuls in fp32r: 11-bit-mantissa fp32, full
PE rate at N=512):
  - S is computed TRANSPOSED (j on partitions, i free) so the softmax sum
    over j becomes a ones-vector matmul and A feeds the second matmul with
    no transposes at all.  Softmax max-subtraction is skipped (S ~ N(0,1),
    exp never overflows in fp32).
  - v is computed transposed (vT[s, c]) directly by the projection matmul
    (h slice as the stationary operand).
  - exp(S^T) is left unnormalized; 1/sum(E) (a per-column factor) is
    broadcast across partitions with a K=1 ones matmul and multiplied into
    the attention output while draining its PSUM.
  - bv folds into an effective output bias bo_eff = bo + wo@bv (host).
  - The residual + bias are fused into the PSUM-drain of the output
    projection (one scalar_tensor_tensor per tile), then stored.
  - GroupNorm group stats are computed per-partition with bn_stats, then
    aggregated AND broadcast back to the channel layout in one matmul with
    a block-diagonal 1/16 matrix.
"""
import numpy as np

import concourse.bacc as bacc
import concourse.mybir as mybir
import concourse.tile as tile
from concourse import bass_utils

F32 = mybir.dt.float32
F32R = mybir.dt.float32r
AF = mybir.ActivationFunctionType
ALU = mybir.AluOpType

B, C, HH, WW = 16, 512, 32, 32
N = HH * WW          # 1024 spatial positions
NCORES = 8
NB = B // NCORES     # batches per core
CCH = C // 128       # 4 channel chunks
SCH = N // 128       # 8 spatial chunks
NIH = N // 512       # 2 free-dim halves
GROUPS = 32
GPC = C // GROUPS    # 16 channels per group
EPS = 1e-6
SCALE = float(C) ** -0.5


def _build_program(reps: int = 1, loop_n: int = 1, x_outside: bool = False, skip_gn: bool = False, den_block: bool = False):
    """reps/loop_n > 1 repeat the whole per-core computation (timing only;
    the output accumulates multiple times and is numerically meaningless).
    loop_n uses a hardware For_i loop around the body."""
    nc = bacc.Bacc("TRN2", target_bir_lowering=False, debug=False)

    x_d = nc.dram_tensor("x", [NB, C, N], F32, kind="ExternalInput").ap()
    w_d = {
        name: nc.dram_tensor(name, [C, C], F32R, kind="ExternalInput").ap()
        for name in ("wmT", "wpT")
    }
    u_d = nc.dram_tensor("uvec", [C], F32R, kind="ExternalInput").ap()
    bo_d = nc.dram_tensor("bo", [C], F32, kind="ExternalInput").ap()
    gamma_d = nc.dram_tensor("gamma", [C], F32, kind="ExternalInput").ap()
    beta_d = nc.dram_tensor("beta", [C], F32, kind="ExternalInput").ap()
    wsel_d = nc.dram_tensor("wsel", [128, 128], F32, kind="ExternalInput").ap()
    ones_d = nc.dram_tensor("onesv", [128], F32R, kind="ExternalInput").ap()
    y_d = nc.dram_tensor("y", [NB, C, N], F32, kind="ExternalOutput").ap()

    with tile.TileContext(nc) as tc:
        with (
            tc.tile_pool(name="const", bufs=1) as cp,
            tc.tile_pool(name="data", bufs=1) as dp,
            tc.tile_pool(name="psum", bufs=7, space="PSUM") as pp,
        ):
            # ---- small constants first (tiny DMAs) ----------------------
            def per_chunk_vec(name, src):
                out = []
                for cc in range(CCH):
                    t = cp.tile([128, 1], F32, name=f"{name}_{cc}")
                    nc.sync.dma_start(out=t, in_=src[cc * 128:(cc + 1) * 128])
                    out.append(t)
                return out

            bo_pl = per_chunk_vec("bov", bo_d)
            u_pl = []
            for cc in range(CCH):
                t = cp.tile([128, 1], F32R, name=f"uv_{cc}")
                nc.sync.dma_start(out=t, in_=u_d[cc * 128:(cc + 1) * 128])
                u_pl.append(t)
            gamma_pl = per_chunk_vec("gammav", gamma_d)
            beta_pl = per_chunk_vec("betav", beta_d)
            wsel = cp.tile([128, 128], F32)
            nc.sync.dma_start(out=wsel, in_=wsel_d)
            ones_t = cp.tile([128, 1], F32R)
            nc.sync.dma_start(out=ones_t, in_=ones_d)
            ones_row = cp.tile([1, 128], F32)
            nc.sync.dma_start(out=ones_row, in_=ones_d.bitcast(F32))
            eps_t = cp.tile([128, 1], F32)
            nc.vector.memset(eps_t, EPS)

            # ---- weights (in first-use order; wo last) -------------------
            wts = {}
            for name in ("wmT", "wpT"):
                for ci in range(CCH):
                    t = cp.tile([128, C], F32R, name=f"{name}_{ci}")
                    wts[(name, ci)] = t

            def load_w(name):
                for ci in range(CCH):
                    nc.sync.dma_start(
                        out=wts[(name, ci)],
                        in_=w_d[name][ci * 128:(ci + 1) * 128, :],
                    )

            preloaded_x = [None]

            def emit_rep(r, load_weights=True):
                p = f"r{r}_"

                h_t = [[
                    dp.tile([128, N], F32R, name=f"{p}h{cc}_b{b}", tag=f"h{cc}",
                            bufs=2)
                    for cc in range(CCH)] for b in range(NB)]

                if preloaded_x[0] is not None:
                    x_t = preloaded_x[0]
                else:
                    x_t = [[
                        dp.tile([128, N], F32, name=f"{p}x{cc}_b{b}",
                                tag=f"x{cc}", bufs=2)
                        for cc in range(CCH)] for b in range(NB)]
                    # x loads (halves, so bn_stats starts at half latency)
                    for b in range(NB):
                        for cc in range(CCH):
                            for ih in range(NIH):
                                nc.sync.dma_start(
                                    out=x_t[b][cc][:, ih * 512:(ih + 1) * 512],
                                    in_=x_d[b, cc * 128:(cc + 1) * 128,
                                            ih * 512:(ih + 1) * 512],
                                )
                if r == 0 and load_weights:
                    load_w("wmT")
                    load_w("wpT")

                # ---- groupnorm (both batches; all Sqrts before all Exps) -
                for b in range(NB):
                    for cc in range(CCH):
                        xc = x_t[b][cc]
                        if skip_gn:
                            nc.vector.tensor_scalar(
                                out=h_t[b][cc], in0=xc, scalar1=1.0,
                                scalar2=0.0, op0=ALU.mult, op1=ALU.add,
                            )
                            continue
                        st6 = dp.tile([128, 2, 6], F32, name=f"{p}st6_{b}_{cc}",
                                      tag="st6", bufs=2)
                        nc.vector.bn_stats(out=st6[:, 0, :], in_=xc[:, 0:512])
                        nc.vector.bn_stats(out=st6[:, 1, :], in_=xc[:, 512:N])
                        mv = dp.tile([128, 2], F32, name=f"{p}mv_{b}_{cc}",
                                     tag="mv", bufs=2)
                        nc.vector.bn_aggr(out=mv, in_=st6)
                        stk = dp.tile([128, 2], F32, name=f"{p}stk_{b}_{cc}",
                                      tag="stk", bufs=2)
                        nc.vector.tensor_copy(out=stk[:, 0:1], in_=mv[:, 0:1])
                        nc.vector.tensor_mul(stk[:, 1:2], mv[:, 0:1], mv[:, 0:1])
                        nc.vector.tensor_add(stk[:, 1:2], stk[:, 1:2], mv[:, 1:2])
                        psg = pp.tile([128, 2], F32, name=f"{p}psg_{b}_{cc}",
                                      tag="stat", bufs=1)
                        nc.tensor.matmul(psg, wsel, stk, start=True, stop=True)
                        g2 = dp.tile([128, 2], F32, name=f"{p}g2_{b}_{cc}",
                                     tag="g2", bufs=2)
                        nc.vector.tensor_copy(out=g2, in_=psg)
                        msq = dp.tile([128, 1], F32, name=f"{p}msq_{b}_{cc}",
                                      tag="msq", bufs=2)
                        nc.vector.tensor_mul(msq, g2[:, 0:1], g2[:, 0:1])
                        var = dp.tile([128, 1], F32, name=f"{p}var_{b}_{cc}",
                                      tag="var", bufs=2)
                        nc.vector.tensor_sub(var, g2[:, 1:2], msq)
                        std = dp.tile([128, 1], F32, name=f"{p}std_{b}_{cc}",
                                      tag="std", bufs=2)
                        nc.scalar.activation(std, var, AF.Sqrt, bias=eps_t)
                        rstd = dp.tile([128, 1], F32, name=f"{p}rstd_{b}_{cc}",
                                       tag="rstd", bufs=2)
                        nc.vector.reciprocal(rstd, std)
                        ac = dp.tile([128, 1], F32, name=f"{p}ac_{b}_{cc}",
                                     tag="ac", bufs=2)
                        nc.vector.tensor_mul(ac, rstd, gamma_pl[cc])
                        bc = dp.tile([128, 1], F32, name=f"{p}bc_{b}_{cc}",
                                     tag="bc", bufs=2)
                        nc.vector.tensor_mul(bc, g2[:, 0:1], ac)
                        nc.vector.tensor_sub(bc, beta_pl[cc], bc)
                        nc.vector.tensor_scalar(
                            out=h_t[b][cc], in0=xc, scalar1=ac, scalar2=bc,
                            op0=ALU.mult, op1=ALU.add,
                        )

                # ---- attention, cross-batch interleaved ------------------
                # PE order: QK(0) S(0) VT(0) den(0) | QK(1) | AV(0) | S(1)
                # VT(1) den(1) | Y(0) | AV(1) Y(1).  b1's projections fill
                # b0's softmax-denominator latency; Y(0) fills b1's.
                g_t, vt_t, e_t, rdb_t, on_t, a_t = {}, {}, {}, {}, {}, {}

                def stage_g(b):
                    # g = (Wq^T Wk)^T h : one projection replaces Q and K.
                    # a[j] = scale*(Wk^T bq) . h_j folds into the exp bias;
                    # the i-dependent bias term cancels in softmax.
                    h = h_t[b]
                    g_t[b] = [dp.tile([128, N], F32R, name=f"{p}g{cc}_b{b}",
                                      tag=f"q{cc}") for cc in range(CCH)]
                    for co in range(CCH):
                        for ih in range(NIH):
                            ps = pp.tile([128, 512], F32, tag="mm",
                                         name=f"{p}ps_g{co}{ih}_b{b}")
                            for ci in range(CCH):
                                nc.tensor.matmul(
                                    ps,
                                    wts[("wmT", ci)][:, co * 128:(co + 1) * 128],
                                    h[ci][:, ih * 512:(ih + 1) * 512],
                                    start=(ci == 0), stop=(ci == CCH - 1),
                                )
                            nc.vector.tensor_copy(
                                out=g_t[b][co][:, ih * 512:(ih + 1) * 512],
                                in_=ps,
                            )
                    # a-term: a_row[1, i] = sum_c u[c] h[c, i]
                    a_row = dp.tile([1, N], F32, name=f"{p}arow_b{b}",
                                    tag="arow", bufs=2)
                    for ih in range(NIH):
                        psa = pp.tile([1, 512], F32, name=f"{p}psa{ih}_b{b}",
                                      tag="stat", bufs=1)
                        for ci in range(CCH):
                            nc.tensor.matmul(
                                psa, u_pl[ci],
                                h[ci][:, ih * 512:(ih + 1) * 512],
                                start=(ci == 0), stop=(ci == CCH - 1),
                            )
                        nc.vector.tensor_copy(
                            out=a_row[:, ih * 512:(ih + 1) * 512], in_=psa
                        )
                    # transpose to per-partition layout [128, SCH]
                    a_t[b] = dp.tile([128, SCH], F32, name=f"{p}aall_b{b}",
                                     tag="aall", bufs=2)
                    for jc in range(SCH):
                        nc.sync.dma_start(
                            out=a_t[b][:, jc:jc + 1],
                            in_=a_row[:, jc * 128:(jc + 1) * 128],
                        )

                def stage_s(b):
                    # ih-outer: after the first 8 groups, the ih=0 halves of
                    # all E chunks exist, so denom(ih=0) overlaps S(ih=1).
                    e_t[b] = [dp.tile([128, N], F32R, name=f"{p}E{jc}_b{b}",
                                      tag=f"E{jc}") for jc in range(SCH)]
                    for ih in range(NIH):
                        for jc in range(SCH):
                            ps = pp.tile([128, 512], F32, tag="mm",
                                         name=f"{p}ps_s{jc}{ih}_b{b}")
                            for ci in range(CCH):
                                nc.tensor.matmul(
                                    ps,
                                    g_t[b][ci][:, jc * 128:(jc + 1) * 128],
                                    h_t[b][ci][:, ih * 512:(ih + 1) * 512],
                                    start=(ci == 0), stop=(ci == CCH - 1),
                                )
                            nc.scalar.activation(
                                out=e_t[b][jc][:, ih * 512:(ih + 1) * 512],
                                in_=ps, func=AF.Exp, scale=SCALE,
                                bias=a_t[b][:, jc:jc + 1],
                            )
                        if not den_block:
                            stage_den_half(b, ih)

                def stage_vt(b):
                    # p = (Wo Wv) h, transposed: fuses the V and output
                    # projections (attention weighting is linear in v).
                    h = h_t[b]
                    vt_t[b] = [dp.tile([128, C], F32R, name=f"{p}pt{sc}_b{b}",
                                       tag=f"vt{sc}") for sc in range(SCH)]
                    for sc in range(SCH):
                        ps = pp.tile([128, 512], F32, tag="mm",
                                     name=f"{p}ps_pt{sc}_b{b}")
                        for ci in range(CCH):
                            nc.tensor.matmul(
                                ps,
                                h[ci][:, sc * 128:(sc + 1) * 128],
                                wts[("wpT", ci)],
                                start=(ci == 0), stop=(ci == CCH - 1),
                            )
                        nc.scalar.copy(out=vt_t[b][sc], in_=ps)

                rd_t = {}

                def stage_den_half(b, ih):
                    if ih == 0:
                        rd_t[b] = dp.tile([1, N], F32, name=f"{p}rd_b{b}",
                                          tag="rd", bufs=2)
                        rdb_t[b] = dp.tile([128, N], F32, name=f"{p}rdb_b{b}",
                                           tag="rdb", bufs=2)
                    rd = rd_t[b]
                    psd = pp.tile([1, 512], F32, name=f"{p}psd{ih}_b{b}",
                                  tag="stat", bufs=1)
                    for jc in range(SCH):
                        nc.tensor.matmul(
                            psd, ones_t,
                            e_t[b][jc][:, ih * 512:(ih + 1) * 512],
                            start=(jc == 0), stop=(jc == SCH - 1),
                        )
                    nc.vector.reciprocal(rd[:, ih * 512:(ih + 1) * 512], psd)
                    psb = pp.tile([128, 512], F32, name=f"{p}psb{ih}_b{b}",
                                  tag="stat", bufs=1)
                    nc.tensor.matmul(
                        psb, ones_row, rd[:, ih * 512:(ih + 1) * 512],
                        start=True, stop=True,
                    )
                    nc.scalar.copy(
                        out=rdb_t[b][:, ih * 512:(ih + 1) * 512], in_=psb
                    )

                def stage_av(b):
                    # out[d,i] = sum_j p[d,j] E[j,i]; drain = x rdenom,
                    # + bo_eff, + residual, then store.
                    for dd in range(CCH):
                        for ih in range(NIH):
                            ps = pp.tile([128, 512], F32, tag="mm",
                                         name=f"{p}ps_av{dd}{ih}_b{b}")
                            for jc in range(SCH):
                                nc.tensor.matmul(
                                    ps,
                                    vt_t[b][jc][:, dd * 128:(dd + 1) * 128],
                                    e_t[b][jc][:, ih * 512:(ih + 1) * 512],
                                    start=(jc == 0), stop=(jc == SCH - 1),
                                )
                            tmp = dp.tile([128, 512], F32, tag="avtmp", bufs=3,
                                          name=f"{p}avtmp{dd}{ih}_b{b}")
                            nc.vector.tensor_mul(
                                tmp, ps, rdb_t[b][:, ih * 512:(ih + 1) * 512],
                            )
                            ysb = dp.tile([128, 512], F32, tag="ysb", bufs=3,
                                          name=f"{p}ysb{dd}{ih}_b{b}")
                            nc.vector.scalar_tensor_tensor(
                                out=ysb, in0=tmp, scalar=bo_pl[dd],
                                in1=x_t[b][dd][:, ih * 512:(ih + 1) * 512],
                                op0=ALU.add, op1=ALU.add,
                            )
                            nc.sync.dma_start(
                                out=y_d[b, dd * 128:(dd + 1) * 128,
                                        ih * 512:(ih + 1) * 512],
                                in_=ysb,
                            )


                stage_g(0)
                stage_s(0)       # includes per-half denominator
                stage_vt(0)
                if den_block:
                    stage_den_half(0, 0)
                    stage_den_half(0, 1)
                stage_g(1)
                stage_av(0)
                stage_s(1)
                stage_vt(1)
                if den_block:
                    stage_den_half(1, 0)
                    stage_den_half(1, 1)
                stage_av(1)

            if loop_n > 1:
                for name in ("wmT", "wpT"):
                    load_w(name)
                if x_outside:
                    xo = [[
                        dp.tile([128, N], F32, name=f"xo{cc}_b{b}",
                                tag=f"x{cc}", bufs=2)
                        for cc in range(CCH)] for b in range(NB)]
                    for b in range(NB):
                        for cc in range(CCH):
                            nc.sync.dma_start(
                                out=xo[b][cc],
                                in_=x_d[b, cc * 128:(cc + 1) * 128, :])
                    preloaded_x[0] = xo
                with tc.For_i(0, loop_n, 1,
                              hint_engines=(mybir.EngineType.PE,)):
                    emit_rep(0, load_weights=False)
            else:
                for r in range(reps):
                    emit_rep(r)

    nc.finalize()
    return nc


_PROGRAM = None


def _program():
    global _PROGRAM
    if _PROGRAM is None:
        _PROGRAM = _build_program()
    return _PROGRAM


def _round_f32r(a: np.ndarray) -> np.ndarray:
    """Round fp32 to fp32r (11-bit mantissa) round-to-nearest-even."""
    u = np.ascontiguousarray(a, dtype=np.float32).view(np.uint32)
    low = u & np.uint32(0x00000FFF)
    base = u & np.uint32(0xFFFFF000)
    lsb = (u >> np.uint32(12)) & np.uint32(1)
    round_up = (low > 0x800) | ((low == 0x800) & (lsb == 1))
    return (base + (round_up.astype(np.uint32) << np.uint32(12))).view(np.float32)


def make_in_maps(hidden_states, norm_gamma, norm_beta, wq, bq, wk, bk, wv, bv,
                 wo, bo):
    x = np.ascontiguousarray(hidden_states, dtype=np.float32).reshape(B, C, N)
    wq64 = np.asarray(wq, np.float64)
    wk64 = np.asarray(wk, np.float64)
    shared = {
        "wmT": _round_f32r((wk64.T @ wq64).astype(np.float32)),
        "wpT": _round_f32r(
            (np.asarray(wv, np.float64).T @ np.asarray(wo, np.float64).T)
            .astype(np.float32)
        ),
        "uvec": _round_f32r(
            (SCALE * (wk64.T @ np.asarray(bq, np.float64))).astype(np.float32)
        ),
        "bo": np.ascontiguousarray(
            np.asarray(bo, np.float32)
            + np.asarray(wo, np.float32) @ np.asarray(bv, np.float32)
        ),
        "gamma": np.ascontiguousarray(norm_gamma, np.float32),
        "beta": np.ascontiguousarray(norm_beta, np.float32),
        "wsel": np.kron(np.eye(128 // GPC, dtype=np.float32),
                        np.full((GPC, GPC), 1.0 / GPC, np.float32)),
        "onesv": np.ones((128,), np.float32),
    }
    return [
        {"x": np.ascontiguousarray(x[c * NB:(c + 1) * NB]), **shared}
        for c in range(NCORES)
    ]


def kernel(hidden_states, norm_gamma, norm_beta, wq, bq, wk, bk, wv, bv, wo, bo):
    nc = _program()
    in_maps = make_in_maps(hidden_states, norm_gamma, norm_beta, wq, bq, wk, bk,
                           wv, bv, wo, bo)
    res = bass_utils.run_bass_kernel_spmd(nc, in_maps, core_ids=list(range(NCORES)))
    out = np.concatenate([res.results[c]["y"] for c in range(NCORES)], axis=0)
    return np.ascontiguousarray(out.reshape(B, C, HH, WW), dtype=np.float32)

